# revision 9
# baseline (speedup 1.0000x reference)
"""Trainium2 Bass kernel for nn_ConditionalLayer (MoE-style conditional FC).

Reference semantics (N=16384 rows, D=512 features, C=8 conditions):
    out[n] = sum_c relu( (x[n] * [cond_ids[n]==c]) @ W_c + b_c )
           = relu(x[n] @ W_{c*} + b_{c*}) + corr_{c*}
where c* = cond_ids[n] and corr_c = sum_{c' != c} relu(b_{c'}) is a
per-condition constant vector (masked-out rows still contribute relu(b_c)).

Strategy (expert-parallel, 8 cores == 8 conditions):
  - Host: group rows by condition (argsort), pad to a common CAP, ship core c
    the transposed row-block xT_c = [D, CAP] in fp16 plus W_c (fp16) and b_c.
  - Device: yT = relu(W_c.T-contract xT + b_c), fp16 matmuls on the 128x128
    PE with fp32 PSUM accumulation; PSUM drains split between VectorE
    (tensor_scalar add+max) and ScalarE (activation relu+bias); fp16 out.
  - Host: scatter rows back, adding corr_c in fp32 during the unshard.

The device program is hand-scheduled (raw per-engine Blocks + counting
semaphores, no TileContext): the Tile event-semaphore machinery adds nothing
but overhead for a static pipeline this regular.  fp16 matmuls use 1024-row
moving operands (2 PSUM banks per group) to halve LDWEIGHTS/instruction
overhead.  Warmup matmuls on a junk tile run during the initial DMA window
so the PE HAM clock gate (1.2 GHz cold / 2.4 GHz warm) is released before
real work.  The NEFF epilogue (runtime semaphore-sync chain, ~8us) starts
when the last store lands, so the tail tile is small (128 rows) and the
last stores go on otherwise-idle HWDGE rings.
"""

import math

import numpy as np

N, D, C = 16384, 512, 8
NCORES = 8
P = 128
KT = D // P  # 4 k-tiles
FT = D // P  # 4 output feature tiles
GMAX = 1024  # max x-tile DMA granularity
MMAX = 512  # max moving free dim per matmul (PSUM bank limit)
NPS = 7  # PSUM rotation banks (bank 7 is the warmup target)
NWARM = 17  # warmup matmuls (256 rows each) to release the PE clock gate

_PROGRAM_CACHE: dict = {}


def _x_tile_sizes(cap: int) -> list:
    """DMA granularity: 128 head (starts PE early), <=1024 mids, 128 tail
    so the final store drain is short. cap is a multiple of 128."""
    if cap >= 1536:
        mid = cap - 256  # head 128 + tail 128
        sizes = [128]
        while mid >= 1024:
            sizes.append(1024)
            mid -= 1024
        if mid:
            sizes.append(mid)
        sizes.append(128)
    else:
        sizes = []
        rem = cap
        while rem:
            s = min(GMAX, rem)
            sizes.append(s)
            rem -= s
    assert sum(sizes) == cap
    return sizes


def _build_program(cap: int):
    import concourse.mybir as mybir
    from concourse import bacc

    f32 = mybir.dt.float32
    f16 = mybir.dt.float16
    relu = mybir.ActivationFunctionType.Relu
    add = mybir.AluOpType.add
    amax = mybir.AluOpType.max

    nc = bacc.Bacc("TRN2", target_bir_lowering=False, debug=False)

    xt = nc.dram_tensor("xt", [D, cap], f16, kind="ExternalInput")
    w5 = nc.dram_tensor("w5", [P, FT, KT, P], f16, kind="ExternalInput")
    b2 = nc.dram_tensor("b2", [P, FT], f32, kind="ExternalInput")
    yt = nc.dram_tensor("yt", [D, cap], f16, kind="ExternalOutput")

    xt_r = xt[:].rearrange("(kt p) r -> p kt r", p=P)  # [128, KT, cap]
    yt_r = yt[:].rearrange("(ft p) r -> p ft r", p=P)  # [128, FT, cap]

    sizes = _x_tile_sizes(cap)
    tiles = []  # (roff, rsz)
    off = 0
    for s in sizes:
        tiles.append((off, s))
        off += s
    ntiles = len(tiles)

    # Static group schedule: one group = one PSUM accumulation (<=512 rows,
    # one ft).  Drains alternate VectorE / ScalarE by parity; PSUM slots
    # rotate over NPS banks.
    groups = []  # (g, tile_idx, ft, coff, csz)
    g = 0
    for t, (roff, rsz) in enumerate(tiles):
        chunks = []
        c0 = 0
        while c0 < rsz:
            cs = min(MMAX, rsz - c0)
            chunks.append((c0, cs))
            c0 += cs
        for ft in range(FT):
            for coff, csz in chunks:
                groups.append((g, t, ft, coff, csz))
                g += 1
    last_group_of_tile = {t: max(g for g, tt, *_ in groups if tt == t) for t in range(ntiles)}

    # Store-ring assignment: alternate tiles between the two HWDGE rings so
    # the final store never queues behind a big one on the same ring.
    store_ring = {t: ("scalar" if t % 2 == 0 else "sync") for t in range(ntiles)}

    # SBUF / PSUM allocations
    w_sb = nc.alloc_sbuf_tensor("w_sb", [P, FT, KT, P], f16)
    b_sb = nc.alloc_sbuf_tensor("b_sb", [P, FT], f32)
    wm = nc.alloc_sbuf_tensor("wm", [P, 256], f16)
    x_sb = [
        nc.alloc_sbuf_tensor(f"x_sb{t}", [P, KT, rsz], f16)
        for t, (roff, rsz) in enumerate(tiles)
    ]
    o_sb = [
        nc.alloc_sbuf_tensor(f"o_sb{t}", [P, FT, rsz], f16)
        for t, (roff, rsz) in enumerate(tiles)
    ]
    ps = [nc.alloc_psum_tensor(f"ps{i}", [P, MMAX], f32) for i in range(NPS)]
    wps = nc.alloc_psum_tensor("wps", [P, 256], f32)

    # One semaphore per load DMA: a threshold of 16 means "all 16 SDMA
    # engines finished this transfer".  (Intermediate thresholds on a shared
    # sem are racy: a later transfer's per-engine increments can land while
    # an earlier one is still in flight.)
    sem_x = [nc.alloc_semaphore(f"sem_x{t}") for t in range(ntiles)]
    sem_w1 = nc.alloc_semaphore("sem_w1")
    sem_w2 = nc.alloc_semaphore("sem_w2")
    sem_b = nc.alloc_semaphore("sem_b")
    sem_wm = nc.alloc_semaphore("sem_wm")  # warmup tile memset done
    sem_mm = nc.alloc_semaphore("sem_mm")  # PSUM groups finished (stop-MMs)
    sem_drv = nc.alloc_semaphore("sem_drv")  # VectorE drains done
    sem_drs = nc.alloc_semaphore("sem_drs")  # ScalarE drains done
    sem_sts = nc.alloc_semaphore("sem_sts")  # sync-ring store completions
    sem_sta = nc.alloc_semaphore("sem_sta")  # scalar-ring store completions

    def owner(gidx):  # (sem, per-engine index) of group gidx's drain
        return (sem_drv, gidx // 2) if gidx % 2 == 0 else (sem_drs, gidx // 2)

    def drains_done_thresholds(gidx):
        """(v, s) = per-engine drain counts once groups 0..gidx all drained."""
        cnt = gidx + 1
        return (cnt + 1) // 2, cnt // 2

    def emit_store(eng, t, sem_done):
        roff, rsz = tiles[t]
        vthr, sthr = drains_done_thresholds(last_group_of_tile[t])
        eng.wait_ge(sem_drv, vthr)
        eng.wait_ge(sem_drs, sthr)
        eng.dma_start(yt_r[:, :, roff : roff + rsz], o_sb[t][:, :, :]).then_inc(
            sem_done, 16
        )

    with nc.Block(no_gpsimd_drain=True) as block:

        @block.sync
        def _(sync):
            # x tile loads, in order, on the SP HWDGE ring
            for t, (roff, rsz) in enumerate(tiles):
                sync.dma_start(
                    x_sb[t][:, :, :], xt_r[:, :, roff : roff + rsz]
                ).then_inc(sem_x[t], 16)
            n = 0
            for t in range(ntiles):
                if store_ring[t] == "sync":
                    emit_store(sync, t, sem_sts)
                    n += 1
            sync.wait_ge(sem_sts, 16 * n)

        @block.scalar
        def _(scalar):
            # b + w loads on the Activation HWDGE ring (concurrent with x)
            scalar.dma_start(b_sb[:, :], b2[:]).then_inc(sem_b, 16)
            scalar.dma_start(w_sb[:, :2], w5[:, :2]).then_inc(sem_w1, 16)
            scalar.dma_start(w_sb[:, 2:], w5[:, 2:]).then_inc(sem_w2, 16)
            # odd-parity PSUM drains: relu(z + b), interleaved with this
            # ring's stores so each store issues as soon as its tile drains
            scalar.wait_ge(sem_b, 16)
            pending = [t for t in range(ntiles) if store_ring[t] == "scalar"]
            for g, t, ft, coff, csz in groups:
                if g % 2 == 1:
                    scalar.wait_ge(sem_mm, g + 1)
                    scalar.activation(
                        o_sb[t][:, ft, coff : coff + csz],
                        ps[g % NPS][:, :csz],
                        relu,
                        bias=b_sb[:, ft : ft + 1],
                    ).then_inc(sem_drs, 1)
                while pending and last_group_of_tile[pending[0]] <= g:
                    emit_store(scalar, pending[0], sem_sta)
                    pending.pop(0)
            for t in pending:
                emit_store(scalar, t, sem_sta)
            nst = sum(1 for t in range(ntiles) if store_ring[t] == "scalar")
            scalar.wait_ge(sem_sta, 16 * nst)

        @block.vector
        def _(vector):
            vector.memset(wm[:, :], 0.25).then_inc(sem_wm, 1)
            # even-parity PSUM drains: (z + b) max 0
            vector.wait_ge(sem_b, 16)
            for g, t, ft, coff, csz in groups:
                if g % 2 != 0:
                    continue
                vector.wait_ge(sem_mm, g + 1)
                vector.tensor_scalar(
                    o_sb[t][:, ft, coff : coff + csz],
                    ps[g % NPS][:, :csz],
                    b_sb[:, ft : ft + 1],
                    0.0,
                    add,
                    amax,
                ).then_inc(sem_drv, 1)

        @block.tensor
        def _(tensor):
            tensor.wait_ge(sem_wm, 1)
            for _ in range(NWARM):
                tensor.matmul(
                    wps[:, :], lhsT=wm[:, :P], rhs=wm[:, :], start=True, stop=True
                )
            cur_t = -1
            for g, t, ft, coff, csz in groups:
                if t != cur_t:
                    tensor.wait_ge(sem_x[t], 16)
                    if cur_t == -1:
                        tensor.wait_ge(sem_w1, 16)
                        tensor.wait_ge(sem_w2, 16)
                    cur_t = t
                pg = g - NPS  # previous occupant of this PSUM slot
                if pg >= 0:
                    dsem, didx = owner(pg)
                    tensor.wait_ge(dsem, didx + 1)
                slot = ps[g % NPS]
                for kt in range(KT):
                    mm = tensor.matmul(
                        slot[:, :csz],
                        lhsT=w_sb[:, ft, kt],
                        rhs=x_sb[t][:, kt, coff : coff + csz],
                        start=(kt == 0),
                        stop=(kt == KT - 1),
                    )
                mm.then_inc(sem_mm, 1)

    nc.compile()
    return nc


def _get_program(cap: int):
    if cap not in _PROGRAM_CACHE:
        _PROGRAM_CACHE[cap] = _build_program(cap)
    return _PROGRAM_CACHE[cap]


def _route(x, cond_ids, W, b):
    """Host-side routing: group rows by condition, build per-core inputs."""
    x = np.ascontiguousarray(np.asarray(x, dtype=np.float32))
    cond_ids = np.asarray(cond_ids, dtype=np.int32)
    W = np.asarray(W, dtype=np.float32)
    b = np.asarray(b, dtype=np.float32)

    counts = np.bincount(cond_ids, minlength=C)
    cap = max(P, math.ceil(counts.max() / P) * P)
    order = np.argsort(cond_ids, kind="stable")
    starts = np.concatenate([[0], np.cumsum(counts)])

    relu_b = np.maximum(b, 0.0)  # [C, D]
    S = relu_b.sum(axis=0)  # [D]

    in_maps = []
    rows_per_core = []
    corrs = []
    for c in range(C):
        rows_c = order[starts[c] : starts[c + 1]]
        rows_per_core.append(rows_c)
        xT_c = np.zeros((D, cap), dtype=np.float16)
        if len(rows_c):
            xT_c[:, : len(rows_c)] = x[rows_c].T
        # w5[p, ft, kt, f2] = W_c[kt*128 + p, ft*128 + f2]
        w5 = np.ascontiguousarray(
            W[c].astype(np.float16).reshape(KT, P, FT, P).transpose(1, 2, 0, 3)
        )
        corrs.append(S - relu_b[c])
        in_maps.append(
            {
                "xt": xT_c,
                "w5": w5,
                "b2": np.ascontiguousarray(b[c].reshape(FT, P).T),
            }
        )
    return in_maps, rows_per_core, corrs, cap


def run(x, cond_ids, W, b, trace: bool = False):
    """Run the kernel; returns (out, BassKernelResults)."""
    try:
        from concourse.bass_utils import run_bass_kernel_spmd
    except ImportError:
        import sys

        sys.path.append("/opt/trn_rl_repo")
        from concourse.bass_utils import run_bass_kernel_spmd

    in_maps, rows_per_core, corrs, cap = _route(x, cond_ids, W, b)
    nc = _get_program(cap)
    res = run_bass_kernel_spmd(
        nc, in_maps, core_ids=list(range(NCORES)), trace=trace
    )

    out = np.empty((len(np.asarray(cond_ids)), D), dtype=np.float32)
    for c in range(C):
        rows_c = rows_per_core[c]
        if len(rows_c):
            out[rows_c] = (
                res.results[c]["yt"][:, : len(rows_c)].T.astype(np.float32)
                + corrs[c][None, :]
            )
    return out, res


def kernel(x, cond_ids, W, b):
    out, _ = run(x, cond_ids, W, b, trace=False)
    return out


# revision 12
# speedup vs baseline: 1.1322x; 1.1322x over previous
"""Trainium2 Bass kernel for nn_ConditionalLayer (MoE-style conditional FC).

Reference semantics (N=16384 rows, D=512 features, C=8 conditions):
    out[n] = sum_c relu( (x[n] * [cond_ids[n]==c]) @ W_c + b_c )
           = relu(x[n] @ W_{c*} + b_{c*}) + corr_{c*}
where c* = cond_ids[n] and corr_c = sum_{c' != c} relu(b_{c'}) is a
per-condition constant vector (masked-out rows still contribute relu(b_c)).

Strategy (expert-parallel, 8 cores == 8 conditions):
  - Host: group rows by condition (argsort), pad to a common CAP, ship core c
    the transposed row-block xT_c = [D, CAP] in fp16 plus W_c (fp16) and b_c.
  - Device: yT = relu(W_c.T-contract xT + b_c), fp16 matmuls on the 128x128
    PE with fp32 PSUM accumulation; PSUM drains split between VectorE
    (tensor_scalar add+max) and ScalarE (activation relu+bias); fp16 out.
  - Host: scatter rows back, adding corr_c in fp32 during the unshard.

The device program is hand-scheduled (raw per-engine Blocks + counting
semaphores, no TileContext): the Tile event-semaphore machinery adds nothing
but overhead for a static pipeline this regular.  fp16 matmuls use 1024-row
moving operands (2 PSUM banks per group) to halve LDWEIGHTS/instruction
overhead.  Warmup matmuls on a junk tile run during the initial DMA window
so the PE HAM clock gate (1.2 GHz cold / 2.4 GHz warm) is released before
real work.  The NEFF epilogue (runtime semaphore-sync chain, ~8us) starts
when the last store lands, so the tail tile is small (128 rows) and the
last stores go on otherwise-idle HWDGE rings.
"""

import math

import numpy as np

N, D, C = 16384, 512, 8
NCORES = 8
P = 128
KT = D // P  # 4 k-tiles
FT = D // P  # 4 output feature tiles
GMAX = 1024  # max x-tile DMA granularity
MMAX = 512  # max moving free dim per matmul (PSUM bank limit)
NPS = 7  # PSUM rotation banks (bank 7 is the warmup target)
NWARM = 24  # warmup matmuls (256 rows each) to release the PE clock gate

_PROGRAM_CACHE: dict = {}


def _x_tile_sizes(cap: int) -> list:
    """DMA granularity: 128 head (starts PE early), <=1024 mids, 128 tail
    so the final store drain is short."""
    if cap >= 1536:
        mid = cap - 256  # head 128 + tail 128
        sizes = [128]
        while mid >= 1024:
            sizes.append(1024)
            mid -= 1024
        if mid:
            sizes.append(mid)
        sizes.append(128)
    else:
        sizes = []
        rem = cap
        while rem:
            s = min(GMAX, rem)
            sizes.append(s)
            rem -= s
    assert sum(sizes) == cap
    return sizes


def _build_program(cap: int):
    import concourse.mybir as mybir
    from concourse import bacc

    f32 = mybir.dt.float32
    f16 = mybir.dt.float16
    relu = mybir.ActivationFunctionType.Relu
    add = mybir.AluOpType.add
    amax = mybir.AluOpType.max

    nc = bacc.Bacc("TRN2", target_bir_lowering=False, debug=False)

    # x and y are packed per tile on the host so every DMA moves one fully
    # contiguous multi-KB line per partition (single descriptor each):
    #   xp[p, KT*roff + kt*rsz + r] = x[kt*128+p, roff+r]
    #   yp[p, FT*roff + ft*rsz + r] = y[ft*128+p, roff+r]
    xp = nc.dram_tensor("xp", [P, KT * cap], f16, kind="ExternalInput")
    w5 = nc.dram_tensor("w5", [P, FT, KT, P], f16, kind="ExternalInput")
    b2 = nc.dram_tensor("b2", [P, FT], f32, kind="ExternalInput")
    yp = nc.dram_tensor("yp", [P, FT * cap], f16, kind="ExternalOutput")

    sizes = _x_tile_sizes(cap)
    tiles = []  # (roff, rsz)
    off = 0
    for s in sizes:
        tiles.append((off, s))
        off += s
    ntiles = len(tiles)

    # Static group schedule: one group = one PSUM accumulation (<=512 rows,
    # one ft).  Drains alternate VectorE / ScalarE by parity; PSUM slots
    # rotate over NPS banks.
    groups = []  # (g, tile_idx, ft, coff, csz)
    g = 0
    for t, (roff, rsz) in enumerate(tiles):
        chunks = []
        c0 = 0
        while c0 < rsz:
            cs = min(MMAX, rsz - c0)
            chunks.append((c0, cs))
            c0 += cs
        for ft in range(FT):
            for coff, csz in chunks:
                groups.append((g, t, ft, coff, csz))
                g += 1
    last_group_of_tile = {t: max(g for g, tt, *_ in groups if tt == t) for t in range(ntiles)}

    # All stores ride the SP HWDGE ring: it is idle once the x loads are
    # triggered, and keeping store triggers off the Activation ring keeps
    # ScalarE's drain queue from stalling behind a ~0.7us DMA trigger.

    # SBUF / PSUM allocations
    w_sb = nc.alloc_sbuf_tensor("w_sb", [P, FT, KT, P], f16)
    b_sb = nc.alloc_sbuf_tensor("b_sb", [P, FT], f32)
    wm = nc.alloc_sbuf_tensor("wm", [P, 256], f16)
    x_sb = [
        nc.alloc_sbuf_tensor(f"x_sb{t}", [P, KT * rsz], f16)
        for t, (roff, rsz) in enumerate(tiles)
    ]
    o_sb = [
        nc.alloc_sbuf_tensor(f"o_sb{t}", [P, FT * rsz], f16)
        for t, (roff, rsz) in enumerate(tiles)
    ]
    ps = [nc.alloc_psum_tensor(f"ps{i}", [P, MMAX], f32) for i in range(NPS)]
    wps = nc.alloc_psum_tensor("wps", [P, 256], f32)

    # One semaphore per load DMA: a threshold of 16 means "all 16 SDMA
    # engines finished this transfer".  (Intermediate thresholds on a shared
    # sem are racy: a later transfer's per-engine increments can land while
    # an earlier one is still in flight.)
    sem_x = [nc.alloc_semaphore(f"sem_x{t}") for t in range(ntiles)]
    sem_w1 = nc.alloc_semaphore("sem_w1")
    sem_w2 = nc.alloc_semaphore("sem_w2")
    sem_b = nc.alloc_semaphore("sem_b")
    sem_wm = nc.alloc_semaphore("sem_wm")  # warmup tile memset done
    sem_mm = nc.alloc_semaphore("sem_mm")  # PSUM groups finished (stop-MMs)
    sem_drv = nc.alloc_semaphore("sem_drv")  # VectorE drains done
    sem_drs = nc.alloc_semaphore("sem_drs")  # ScalarE drains done
    sem_sts = nc.alloc_semaphore("sem_sts")  # store completions (unwaited)

    def owner(gidx):  # (sem, per-engine index) of group gidx's drain
        return (sem_drv, gidx // 2) if gidx % 2 == 0 else (sem_drs, gidx // 2)

    def drains_done_thresholds(gidx):
        """(v, s) = per-engine drain counts once groups 0..gidx all drained."""
        cnt = gidx + 1
        return (cnt + 1) // 2, cnt // 2

    def emit_store(eng, t, sem_done):
        roff, rsz = tiles[t]
        vthr, sthr = drains_done_thresholds(last_group_of_tile[t])
        eng.wait_ge(sem_drv, vthr)
        eng.wait_ge(sem_drs, sthr)
        eng.dma_start(
            yp[:, FT * roff : FT * (roff + rsz)], o_sb[t][:, :]
        ).then_inc(sem_done, 16)

    with nc.Block(no_gpsimd_drain=True) as block:

        @block.sync
        def _(sync):
            # x tile loads, in order, on the SP HWDGE ring
            for t, (roff, rsz) in enumerate(tiles):
                sync.dma_start(
                    x_sb[t][:, :], xp[:, KT * roff : KT * (roff + rsz)]
                ).then_inc(sem_x[t], 16)
            for t in range(ntiles):
                emit_store(sync, t, sem_sts)
            # no final wait: the Block-exit InstDrain on this engine blocks
            # until its DMA queues (including these stores) fully complete

        @block.scalar
        def _(scalar):
            # b + w loads on the Activation HWDGE ring (concurrent with x)
            scalar.dma_start(b_sb[:, :], b2[:]).then_inc(sem_b, 16)
            scalar.dma_start(w_sb[:, :2], w5[:, :2]).then_inc(sem_w1, 16)
            scalar.dma_start(w_sb[:, 2:], w5[:, 2:]).then_inc(sem_w2, 16)
            # odd-parity PSUM drains: relu(z + b), interleaved with this
            # ring's stores so each store issues as soon as its tile drains
            scalar.wait_ge(sem_b, 16)
            for g, t, ft, coff, csz in groups:
                if g % 2 != 1:
                    continue
                scalar.wait_ge(sem_mm, g + 1)
                rsz = tiles[t][1]
                scalar.activation(
                    o_sb[t][:, ft * rsz + coff : ft * rsz + coff + csz],
                    ps[g % NPS][:, :csz],
                    relu,
                    bias=b_sb[:, ft : ft + 1],
                ).then_inc(sem_drs, 1)

        @block.vector
        def _(vector):
            vector.memset(wm[:, :], 0.25).then_inc(sem_wm, 1)
            # even-parity PSUM drains: (z + b) max 0
            vector.wait_ge(sem_b, 16)
            for g, t, ft, coff, csz in groups:
                if g % 2 != 0:
                    continue
                vector.wait_ge(sem_mm, g + 1)
                rsz = tiles[t][1]
                vector.tensor_scalar(
                    o_sb[t][:, ft * rsz + coff : ft * rsz + coff + csz],
                    ps[g % NPS][:, :csz],
                    b_sb[:, ft : ft + 1],
                    0.0,
                    add,
                    amax,
                ).then_inc(sem_drv, 1)

        @block.tensor
        def _(tensor):
            tensor.wait_ge(sem_wm, 1)
            for _ in range(NWARM):
                tensor.matmul(
                    wps[:, :], lhsT=wm[:, :P], rhs=wm[:, :], start=True, stop=True
                )
            cur_t = -1
            for g, t, ft, coff, csz in groups:
                if t != cur_t:
                    tensor.wait_ge(sem_x[t], 16)
                    if cur_t == -1:
                        tensor.wait_ge(sem_w1, 16)
                        tensor.wait_ge(sem_w2, 16)
                    cur_t = t
                pg = g - NPS  # previous occupant of this PSUM slot
                if pg >= 0:
                    dsem, didx = owner(pg)
                    tensor.wait_ge(dsem, didx + 1)
                slot = ps[g % NPS]
                rsz = tiles[t][1]
                for kt in range(KT):
                    mm = tensor.matmul(
                        slot[:, :csz],
                        lhsT=w_sb[:, ft, kt],
                        rhs=x_sb[t][:, kt * rsz + coff : kt * rsz + coff + csz],
                        start=(kt == 0),
                        stop=(kt == KT - 1),
                    )
                mm.then_inc(sem_mm, 1)

    nc.compile()
    return nc


def _get_program(cap: int):
    if cap not in _PROGRAM_CACHE:
        _PROGRAM_CACHE[cap] = _build_program(cap)
    return _PROGRAM_CACHE[cap]


def _route(x, cond_ids, W, b):
    """Host-side routing: group rows by condition, build per-core inputs."""
    x = np.ascontiguousarray(np.asarray(x, dtype=np.float32))
    cond_ids = np.asarray(cond_ids, dtype=np.int32)
    W = np.asarray(W, dtype=np.float32)
    b = np.asarray(b, dtype=np.float32)

    counts = np.bincount(cond_ids, minlength=C)
    # exact cap (8-aligned): tile/matmul free dims may be arbitrary, so no
    # 128-padding — padded rows would be pure wasted PE/DMA work
    cap = max(P, math.ceil(counts.max() / 8) * 8)
    order = np.argsort(cond_ids, kind="stable")
    starts = np.concatenate([[0], np.cumsum(counts)])

    relu_b = np.maximum(b, 0.0)  # [C, D]
    S = relu_b.sum(axis=0)  # [D]

    tiles = []
    off = 0
    for s in _x_tile_sizes(cap):
        tiles.append((off, s))
        off += s

    in_maps = []
    rows_per_core = []
    corrs = []
    for c in range(C):
        rows_c = order[starts[c] : starts[c + 1]]
        rows_per_core.append(rows_c)
        xT_c = np.zeros((D, cap), dtype=np.float16)
        if len(rows_c):
            xT_c[:, : len(rows_c)] = x[rows_c].T
        # pack per tile: xp[p, KT*roff + kt*rsz + r] = xT_c[kt*128+p, roff+r]
        xT_k = xT_c.reshape(KT, P, cap)
        xp = np.empty((P, KT * cap), dtype=np.float16)
        for roff, rsz in tiles:
            blk = xT_k[:, :, roff : roff + rsz].transpose(1, 0, 2)
            xp[:, KT * roff : KT * (roff + rsz)] = blk.reshape(P, KT * rsz)
        # w5[p, ft, kt, f2] = W_c[kt*128 + p, ft*128 + f2]
        w5 = np.ascontiguousarray(
            W[c].astype(np.float16).reshape(KT, P, FT, P).transpose(1, 2, 0, 3)
        )
        corrs.append(S - relu_b[c])
        in_maps.append(
            {
                "xp": xp,
                "w5": w5,
                "b2": np.ascontiguousarray(b[c].reshape(FT, P).T),
            }
        )
    return in_maps, rows_per_core, corrs, cap, tiles


def run(x, cond_ids, W, b, trace: bool = False):
    """Run the kernel; returns (out, BassKernelResults)."""
    try:
        from concourse.bass_utils import run_bass_kernel_spmd
    except ImportError:
        import sys

        sys.path.append("/opt/trn_rl_repo")
        from concourse.bass_utils import run_bass_kernel_spmd

    in_maps, rows_per_core, corrs, cap, tiles = _route(x, cond_ids, W, b)
    nc = _get_program(cap)
    res = run_bass_kernel_spmd(
        nc, in_maps, core_ids=list(range(NCORES)), trace=trace
    )

    out = np.empty((len(np.asarray(cond_ids)), D), dtype=np.float32)
    yT = np.empty((D, cap), dtype=np.float16)
    yT_k = yT.reshape(FT, P, cap)
    for c in range(C):
        rows_c = rows_per_core[c]
        if not len(rows_c):
            continue
        yp = res.results[c]["yp"]
        for roff, rsz in tiles:
            blk = yp[:, FT * roff : FT * (roff + rsz)].reshape(P, FT, rsz)
            yT_k[:, :, roff : roff + rsz] = blk.transpose(1, 0, 2)
        out[rows_c] = (
            yT[:, : len(rows_c)].T.astype(np.float32) + corrs[c][None, :]
        )
    return out, res


def kernel(x, cond_ids, W, b):
    out, _ = run(x, cond_ids, W, b, trace=False)
    return out


# revision 13
# speedup vs baseline: 1.2212x; 1.0786x over previous
"""Trainium2 Bass kernel for nn_ConditionalLayer (MoE-style conditional FC).

Reference semantics (N=16384 rows, D=512 features, C=8 conditions):
    out[n] = sum_c relu( (x[n] * [cond_ids[n]==c]) @ W_c + b_c )
           = relu(x[n] @ W_{c*} + b_{c*}) + corr_{c*}
where c* = cond_ids[n] and corr_c = sum_{c' != c} relu(b_{c'}) is a
per-condition constant vector (masked-out rows still contribute relu(b_c)).

Strategy (expert-parallel, 8 cores == 8 conditions):
  - Host: group rows by condition (argsort), pad to a common CAP, ship core c
    the transposed row-block xT_c = [D, CAP] in fp16 plus W_c (fp16) and b_c.
  - Device: yT = relu(W_c.T-contract xT + b_c), fp16 matmuls on the 128x128
    PE with fp32 PSUM accumulation; PSUM drains split between VectorE
    (tensor_scalar add+max) and ScalarE (activation relu+bias); fp16 out.
  - Host: scatter rows back, adding corr_c in fp32 during the unshard.

The device program is hand-scheduled (raw per-engine Blocks + counting
semaphores, no TileContext): the Tile event-semaphore machinery adds nothing
but overhead for a static pipeline this regular.  fp16 matmuls use 1024-row
moving operands (2 PSUM banks per group) to halve LDWEIGHTS/instruction
overhead.  Warmup matmuls on a junk tile run during the initial DMA window
so the PE HAM clock gate (1.2 GHz cold / 2.4 GHz warm) is released before
real work.  The NEFF epilogue (runtime semaphore-sync chain, ~8us) starts
when the last store lands, so the tail tile is small (128 rows) and the
last stores go on otherwise-idle HWDGE rings.
"""

import math

import numpy as np

N, D, C = 16384, 512, 8
NCORES = 8
P = 128
KT = D // P  # 4 k-tiles
FT = D // P  # 4 output feature tiles
GMAX = 1024  # max x-tile DMA granularity
MMAX = 512  # max moving free dim per matmul (PSUM bank limit)
NPS = 7  # PSUM rotation banks (bank 7 is the warmup target)
NWARM = 24  # warmup matmuls (256 rows each) to release the PE clock gate

_PROGRAM_CACHE: dict = {}


def _x_tile_sizes(cap: int) -> list:
    """DMA granularity: 128 head (starts PE early), <=1024 mids, 128 tail
    so the final store drain is short."""
    if cap >= 1536:
        mid = cap - 256  # head 128 + tail 128
        sizes = [128]
        while mid >= 1024:
            sizes.append(1024)
            mid -= 1024
        if mid:
            sizes.append(mid)
        sizes.append(128)
    else:
        sizes = []
        rem = cap
        while rem:
            s = min(GMAX, rem)
            sizes.append(s)
            rem -= s
    assert sum(sizes) == cap
    return sizes


def _build_program(cap: int):
    import concourse.mybir as mybir
    from concourse import bacc

    f32 = mybir.dt.float32
    f16 = mybir.dt.float16
    relu = mybir.ActivationFunctionType.Relu
    add = mybir.AluOpType.add
    amax = mybir.AluOpType.max

    nc = bacc.Bacc("TRN2", target_bir_lowering=False, debug=False)

    xt = nc.dram_tensor("xt", [D, cap], f16, kind="ExternalInput")
    w5 = nc.dram_tensor("w5", [P, FT, KT, P], f16, kind="ExternalInput")
    b2 = nc.dram_tensor("b2", [P, FT], f32, kind="ExternalInput")
    yt = nc.dram_tensor("yt", [D, cap], f16, kind="ExternalOutput")

    xt_r = xt[:].rearrange("(kt p) r -> p kt r", p=P)  # [128, KT, cap]
    yt_r = yt[:].rearrange("(ft p) r -> p ft r", p=P)  # [128, FT, cap]

    sizes = _x_tile_sizes(cap)
    tiles = []  # (roff, rsz)
    off = 0
    for s in sizes:
        tiles.append((off, s))
        off += s
    ntiles = len(tiles)

    # Static group schedule: one group = one PSUM accumulation (<=512 rows,
    # one ft).  Drains alternate VectorE / ScalarE by parity; PSUM slots
    # rotate over NPS banks.
    groups = []  # (g, tile_idx, ft, coff, csz)
    g = 0
    for t, (roff, rsz) in enumerate(tiles):
        chunks = []
        c0 = 0
        while c0 < rsz:
            cs = min(MMAX, rsz - c0)
            chunks.append((c0, cs))
            c0 += cs
        for ft in range(FT):
            for coff, csz in chunks:
                groups.append((g, t, ft, coff, csz))
                g += 1
    last_group_of_tile = {t: max(g for g, tt, *_ in groups if tt == t) for t in range(ntiles)}

    # All stores ride the SP HWDGE ring: it is idle once the x loads are
    # triggered, and keeping store triggers off the Activation ring keeps
    # ScalarE's drain queue from stalling behind a ~0.7us DMA trigger.

    # SBUF / PSUM allocations
    w_sb = nc.alloc_sbuf_tensor("w_sb", [P, FT, KT, P], f16)
    b_sb = nc.alloc_sbuf_tensor("b_sb", [P, FT], f32)
    wm = nc.alloc_sbuf_tensor("wm", [P, 256], f16)
    x_sb = [
        nc.alloc_sbuf_tensor(f"x_sb{t}", [P, KT, rsz], f16)
        for t, (roff, rsz) in enumerate(tiles)
    ]
    o_sb = [
        nc.alloc_sbuf_tensor(f"o_sb{t}", [P, FT, rsz], f16)
        for t, (roff, rsz) in enumerate(tiles)
    ]
    ps = [nc.alloc_psum_tensor(f"ps{i}", [P, MMAX], f32) for i in range(NPS)]
    wps = nc.alloc_psum_tensor("wps", [P, 256], f32)

    # One semaphore per load DMA: a threshold of 16 means "all 16 SDMA
    # engines finished this transfer".  (Intermediate thresholds on a shared
    # sem are racy: a later transfer's per-engine increments can land while
    # an earlier one is still in flight.)
    sem_x = [nc.alloc_semaphore(f"sem_x{t}") for t in range(ntiles)]
    sem_wf = [nc.alloc_semaphore(f"sem_wf{f}") for f in range(FT)]
    sem_b = nc.alloc_semaphore("sem_b")
    sem_wm = nc.alloc_semaphore("sem_wm")  # warmup tile memset done
    sem_mm = nc.alloc_semaphore("sem_mm")  # PSUM groups finished (stop-MMs)
    sem_drv = nc.alloc_semaphore("sem_drv")  # VectorE drains done
    sem_drs = nc.alloc_semaphore("sem_drs")  # ScalarE drains done
    sem_sts = nc.alloc_semaphore("sem_sts")  # store completions (unwaited)

    def owner(gidx):  # (sem, per-engine index) of group gidx's drain
        return (sem_drv, gidx // 2) if gidx % 2 == 0 else (sem_drs, gidx // 2)

    def drains_done_thresholds(gidx):
        """(v, s) = per-engine drain counts once groups 0..gidx all drained."""
        cnt = gidx + 1
        return (cnt + 1) // 2, cnt // 2

    def emit_store(eng, t, sem_done):
        roff, rsz = tiles[t]
        vthr, sthr = drains_done_thresholds(last_group_of_tile[t])
        eng.wait_ge(sem_drv, vthr)
        eng.wait_ge(sem_drs, sthr)
        eng.dma_start(yt_r[:, :, roff : roff + rsz], o_sb[t][:, :, :]).then_inc(
            sem_done, 16
        )

    with nc.Block(no_gpsimd_drain=True) as block:

        @block.sync
        def _(sync):
            # First wave balanced across the two HWDGE rings: this ring
            # carries x0 + the first two W ft-chunks, the Activation ring
            # carries b + the other two — the first matmul group is gated on
            # whichever ring finishes its share, so neither should hog the
            # whole first wave.
            for t, (roff, rsz) in enumerate(tiles):
                sync.dma_start(
                    x_sb[t][:, :, :], xt_r[:, :, roff : roff + rsz]
                ).then_inc(sem_x[t], 16)
                if t == 0:
                    sync.dma_start(w_sb[:, 0], w5[:, 0]).then_inc(sem_wf[0], 16)
                    sync.dma_start(w_sb[:, 1], w5[:, 1]).then_inc(sem_wf[1], 16)
            for t in range(ntiles):
                emit_store(sync, t, sem_sts)
            # no final wait: the Block-exit InstDrain on this engine blocks
            # until its DMA queues (including these stores) fully complete

        @block.scalar
        def _(scalar):
            # b + second half of W on the Activation HWDGE ring
            scalar.dma_start(b_sb[:, :], b2[:]).then_inc(sem_b, 16)
            scalar.dma_start(w_sb[:, 2], w5[:, 2]).then_inc(sem_wf[2], 16)
            scalar.dma_start(w_sb[:, 3], w5[:, 3]).then_inc(sem_wf[3], 16)
            # odd-parity PSUM drains: relu(z + b), interleaved with this
            # ring's stores so each store issues as soon as its tile drains
            scalar.wait_ge(sem_b, 16)
            for g, t, ft, coff, csz in groups:
                if g % 2 != 1:
                    continue
                scalar.wait_ge(sem_mm, g + 1)
                scalar.activation(
                    o_sb[t][:, ft, coff : coff + csz],
                    ps[g % NPS][:, :csz],
                    relu,
                    bias=b_sb[:, ft : ft + 1],
                ).then_inc(sem_drs, 1)

        @block.vector
        def _(vector):
            vector.memset(wm[:, :], 0.25).then_inc(sem_wm, 1)
            # even-parity PSUM drains: (z + b) max 0
            vector.wait_ge(sem_b, 16)
            for g, t, ft, coff, csz in groups:
                if g % 2 != 0:
                    continue
                vector.wait_ge(sem_mm, g + 1)
                vector.tensor_scalar(
                    o_sb[t][:, ft, coff : coff + csz],
                    ps[g % NPS][:, :csz],
                    b_sb[:, ft : ft + 1],
                    0.0,
                    add,
                    amax,
                ).then_inc(sem_drv, 1)

        @block.tensor
        def _(tensor):
            tensor.wait_ge(sem_wm, 1)
            for _ in range(NWARM):
                tensor.matmul(
                    wps[:, :], lhsT=wm[:, :P], rhs=wm[:, :], start=True, stop=True
                )
            cur_t = -1
            w_waited = [False] * FT
            for g, t, ft, coff, csz in groups:
                if t != cur_t:
                    tensor.wait_ge(sem_x[t], 16)
                    cur_t = t
                if not w_waited[ft]:
                    tensor.wait_ge(sem_wf[ft], 16)
                    w_waited[ft] = True
                pg = g - NPS  # previous occupant of this PSUM slot
                if pg >= 0:
                    dsem, didx = owner(pg)
                    tensor.wait_ge(dsem, didx + 1)
                slot = ps[g % NPS]
                for kt in range(KT):
                    mm = tensor.matmul(
                        slot[:, :csz],
                        lhsT=w_sb[:, ft, kt],
                        rhs=x_sb[t][:, kt, coff : coff + csz],
                        start=(kt == 0),
                        stop=(kt == KT - 1),
                    )
                mm.then_inc(sem_mm, 1)

    nc.compile()
    return nc


def _get_program(cap: int):
    if cap not in _PROGRAM_CACHE:
        _PROGRAM_CACHE[cap] = _build_program(cap)
    return _PROGRAM_CACHE[cap]


def _route(x, cond_ids, W, b):
    """Host-side routing: group rows by condition, build per-core inputs."""
    x = np.ascontiguousarray(np.asarray(x, dtype=np.float32))
    cond_ids = np.asarray(cond_ids, dtype=np.int32)
    W = np.asarray(W, dtype=np.float32)
    b = np.asarray(b, dtype=np.float32)

    counts = np.bincount(cond_ids, minlength=C)
    # exact cap (8-aligned): tile/matmul free dims may be arbitrary, so no
    # 128-padding — padded rows would be pure wasted PE/DMA work
    cap = max(P, math.ceil(counts.max() / 8) * 8)
    order = np.argsort(cond_ids, kind="stable")
    starts = np.concatenate([[0], np.cumsum(counts)])

    relu_b = np.maximum(b, 0.0)  # [C, D]
    S = relu_b.sum(axis=0)  # [D]

    in_maps = []
    rows_per_core = []
    corrs = []
    for c in range(C):
        rows_c = order[starts[c] : starts[c + 1]]
        rows_per_core.append(rows_c)
        xT_c = np.zeros((D, cap), dtype=np.float16)
        if len(rows_c):
            xT_c[:, : len(rows_c)] = x[rows_c].T
        # w5[p, ft, kt, f2] = W_c[kt*128 + p, ft*128 + f2]
        w5 = np.ascontiguousarray(
            W[c].astype(np.float16).reshape(KT, P, FT, P).transpose(1, 2, 0, 3)
        )
        corrs.append(S - relu_b[c])
        in_maps.append(
            {
                "xt": xT_c,
                "w5": w5,
                "b2": np.ascontiguousarray(b[c].reshape(FT, P).T),
            }
        )
    return in_maps, rows_per_core, corrs, cap


def run(x, cond_ids, W, b, trace: bool = False):
    """Run the kernel; returns (out, BassKernelResults)."""
    try:
        from concourse.bass_utils import run_bass_kernel_spmd
    except ImportError:
        import sys

        sys.path.append("/opt/trn_rl_repo")
        from concourse.bass_utils import run_bass_kernel_spmd

    in_maps, rows_per_core, corrs, cap = _route(x, cond_ids, W, b)
    nc = _get_program(cap)
    res = run_bass_kernel_spmd(
        nc, in_maps, core_ids=list(range(NCORES)), trace=trace
    )

    out = np.empty((len(np.asarray(cond_ids)), D), dtype=np.float32)
    for c in range(C):
        rows_c = rows_per_core[c]
        if len(rows_c):
            out[rows_c] = (
                res.results[c]["yt"][:, : len(rows_c)].T.astype(np.float32)
                + corrs[c][None, :]
            )
    return out, res


def kernel(x, cond_ids, W, b):
    out, _ = run(x, cond_ids, W, b, trace=False)
    return out


# revision 15
# speedup vs baseline: 1.3952x; 1.1425x over previous
"""Trainium2 Bass kernel for nn_ConditionalLayer (MoE-style conditional FC).

Reference semantics (N=16384 rows, D=512 features, C=8 conditions):
    out[n] = sum_c relu( (x[n] * [cond_ids[n]==c]) @ W_c + b_c )
           = relu(x[n] @ W_{c*} + b_{c*}) + corr_{c*}
where c* = cond_ids[n] and corr_c = sum_{c' != c} relu(b_{c'}) is a
per-condition constant vector (masked-out rows still contribute relu(b_c)).

Strategy (expert-parallel, 8 cores == 8 conditions):
  - Host: group rows by condition (argsort), pad to a common CAP, ship core c
    the transposed row-block xT_c = [D, CAP] in fp16 plus W_c (fp16) and b_c.
  - Device: yT = relu(W_c.T-contract xT + b_c), fp16 matmuls on the 128x128
    PE with fp32 PSUM accumulation; PSUM drains split between VectorE
    (tensor_scalar add+max) and ScalarE (activation relu+bias); fp16 out.
  - Host: scatter rows back, adding corr_c in fp32 during the unshard.

The device program is hand-scheduled (raw per-engine Blocks + counting
semaphores, no TileContext): the Tile event-semaphore machinery adds nothing
but overhead for a static pipeline this regular.  Warmup matmuls on a junk
tile run during the initial DMA window so the PE HAM clock gate (1.2 GHz
cold / 2.4 GHz warm) is released before real work.  The first matmul is
gated on the first x tile plus W, so the first wave is balanced across both
HWDGE rings (x0 + W ft0/ft1 on SP, b + W ft2/ft3 on Activation) and kept
small (128-row head tile, 128KB W chunks).  The NEFF epilogue (runtime
semaphore-sync chain, ~8us) starts when the last store lands, so the tail
tile is small (128 rows) and stores ride the SP ring, which is idle after
the loads; the Block-exit drain covers store completion (no explicit final
wait).  CAP is exact (8-aligned, not 128-padded) — matmul free dims and DMA
line lengths may be arbitrary.
"""

import math

import numpy as np

N, D, C = 16384, 512, 8
NCORES = 8
P = 128
KT = D // P  # 4 k-tiles
FT = D // P  # 4 output feature tiles
GMAX = 1024  # max x-tile DMA granularity
MMAX = 512  # max moving free dim per matmul (PSUM bank limit)
NPS = 7  # PSUM rotation banks (bank 7 is the warmup target)
NWARM = 24  # warmup matmuls (256 rows each) to release the PE clock gate

_PROGRAM_CACHE: dict = {}


def _x_tile_sizes(cap: int) -> list:
    """DMA granularity: 128 head (starts PE early), <=1024 mids, 128 tail
    so the final store drain is short."""
    if cap >= 1536:
        mid = cap - 256  # head 128 + tail 128
        sizes = [128]
        while mid >= 1024:
            sizes.append(512)
            sizes.append(512)
            mid -= 1024
        if mid:
            sizes.append(mid)
        sizes.append(128)
    else:
        sizes = []
        rem = cap
        while rem:
            s = min(GMAX, rem)
            sizes.append(s)
            rem -= s
    assert sum(sizes) == cap
    return sizes


def _build_program(cap: int):
    import concourse.mybir as mybir
    from concourse import bacc

    f32 = mybir.dt.float32
    f16 = mybir.dt.float16
    relu = mybir.ActivationFunctionType.Relu
    add = mybir.AluOpType.add
    amax = mybir.AluOpType.max

    nc = bacc.Bacc("TRN2", target_bir_lowering=False, debug=False)

    xt = nc.dram_tensor("xt", [D, cap], f16, kind="ExternalInput")
    w5 = nc.dram_tensor("w5", [P, FT, KT, P], f16, kind="ExternalInput")
    b2 = nc.dram_tensor("b2", [P, FT], f32, kind="ExternalInput")
    yt = nc.dram_tensor("yt", [D, cap], f16, kind="ExternalOutput")

    xt_r = xt[:].rearrange("(kt p) r -> p kt r", p=P)  # [128, KT, cap]
    yt_r = yt[:].rearrange("(ft p) r -> p ft r", p=P)  # [128, FT, cap]

    sizes = _x_tile_sizes(cap)
    tiles = []  # (roff, rsz)
    off = 0
    for s in sizes:
        tiles.append((off, s))
        off += s
    ntiles = len(tiles)

    # Static group schedule: one group = one PSUM accumulation (<=512 rows,
    # one ft).  Drains alternate VectorE / ScalarE by parity; PSUM slots
    # rotate over NPS banks.
    groups = []  # (g, tile_idx, ft, coff, csz)
    g = 0
    for t, (roff, rsz) in enumerate(tiles):
        chunks = []
        c0 = 0
        while c0 < rsz:
            cs = min(MMAX, rsz - c0)
            chunks.append((c0, cs))
            c0 += cs
        # ft order 0,2,1,3: ft0/ft1 weight chunks ride the x ring, ft2/ft3
        # the other — interleaving the rings hides each chunk's arrival
        for ft in (0, 2, 1, 3):
            for coff, csz in chunks:
                groups.append((g, t, ft, coff, csz))
                g += 1
    last_group_of_tile = {t: max(g for g, tt, *_ in groups if tt == t) for t in range(ntiles)}

    # All stores ride the SP HWDGE ring: it is idle once the x loads are
    # triggered, and keeping store triggers off the Activation ring keeps
    # ScalarE's drain queue from stalling behind a ~0.7us DMA trigger.

    # SBUF / PSUM allocations
    w_sb = nc.alloc_sbuf_tensor("w_sb", [P, FT, KT, P], f16)
    b_sb = nc.alloc_sbuf_tensor("b_sb", [P, FT], f32)
    wm = nc.alloc_sbuf_tensor("wm", [P, 256], f16)
    x_sb = [
        nc.alloc_sbuf_tensor(f"x_sb{t}", [P, KT, rsz], f16)
        for t, (roff, rsz) in enumerate(tiles)
    ]
    o_sb = [
        nc.alloc_sbuf_tensor(f"o_sb{t}", [P, FT, rsz], f16)
        for t, (roff, rsz) in enumerate(tiles)
    ]
    ps = [nc.alloc_psum_tensor(f"ps{i}", [P, MMAX], f32) for i in range(NPS)]
    wps = nc.alloc_psum_tensor("wps", [P, 256], f32)

    # One semaphore per load DMA: a threshold of 16 means "all 16 SDMA
    # engines finished this transfer".  (Intermediate thresholds on a shared
    # sem are racy: a later transfer's per-engine increments can land while
    # an earlier one is still in flight.)
    sem_x = [nc.alloc_semaphore(f"sem_x{t}") for t in range(ntiles)]
    sem_wf = [nc.alloc_semaphore(f"sem_wf{f}") for f in range(FT)]
    sem_b = nc.alloc_semaphore("sem_b")
    sem_wm = nc.alloc_semaphore("sem_wm")  # warmup tile memset done
    sem_mm = nc.alloc_semaphore("sem_mm")  # PSUM groups finished (stop-MMs)
    sem_drv = nc.alloc_semaphore("sem_drv")  # VectorE drains done
    sem_drs = nc.alloc_semaphore("sem_drs")  # ScalarE drains done
    sem_sts = nc.alloc_semaphore("sem_sts")  # store completions (unwaited)

    def owner(gidx):  # (sem, per-engine index) of group gidx's drain
        return (sem_drv, gidx // 2) if gidx % 2 == 0 else (sem_drs, gidx // 2)

    def drains_done_thresholds(gidx):
        """(v, s) = per-engine drain counts once groups 0..gidx all drained."""
        cnt = gidx + 1
        return (cnt + 1) // 2, cnt // 2

    def emit_store(eng, t, sem_done):
        roff, rsz = tiles[t]
        vthr, sthr = drains_done_thresholds(last_group_of_tile[t])
        eng.wait_ge(sem_drv, vthr)
        eng.wait_ge(sem_drs, sthr)
        eng.dma_start(yt_r[:, :, roff : roff + rsz], o_sb[t][:, :, :]).then_inc(
            sem_done, 16
        )

    with nc.Block(no_gpsimd_drain=True) as block:

        @block.sync
        def _(sync):
            # First wave balanced across the two HWDGE rings: this ring
            # carries x0 + the first two W ft-chunks, the Activation ring
            # carries b + the other two — the first matmul group is gated on
            # whichever ring finishes its share, so neither should hog the
            # whole first wave.
            for t, (roff, rsz) in enumerate(tiles):
                sync.dma_start(
                    x_sb[t][:, :, :], xt_r[:, :, roff : roff + rsz]
                ).then_inc(sem_x[t], 16)
                if t == 0:
                    sync.dma_start(w_sb[:, 0], w5[:, 0]).then_inc(sem_wf[0], 16)
                    sync.dma_start(w_sb[:, 1], w5[:, 1]).then_inc(sem_wf[1], 16)
            for t in range(ntiles):
                emit_store(sync, t, sem_sts)
            # no final wait: the Block-exit InstDrain on this engine blocks
            # until its DMA queues (including these stores) fully complete

        @block.scalar
        def _(scalar):
            # b + second half of W on the Activation HWDGE ring
            scalar.dma_start(b_sb[:, :], b2[:]).then_inc(sem_b, 16)
            scalar.dma_start(w_sb[:, 2], w5[:, 2]).then_inc(sem_wf[2], 16)
            scalar.dma_start(w_sb[:, 3], w5[:, 3]).then_inc(sem_wf[3], 16)
            # odd-parity PSUM drains: relu(z + b), interleaved with this
            # ring's stores so each store issues as soon as its tile drains
            scalar.wait_ge(sem_b, 16)
            for g, t, ft, coff, csz in groups:
                if g % 2 != 1:
                    continue
                scalar.wait_ge(sem_mm, g + 1)
                scalar.activation(
                    o_sb[t][:, ft, coff : coff + csz],
                    ps[g % NPS][:, :csz],
                    relu,
                    bias=b_sb[:, ft : ft + 1],
                ).then_inc(sem_drs, 1)

        @block.vector
        def _(vector):
            vector.memset(wm[:, :], 0.25).then_inc(sem_wm, 1)
            # even-parity PSUM drains: (z + b) max 0
            vector.wait_ge(sem_b, 16)
            for g, t, ft, coff, csz in groups:
                if g % 2 != 0:
                    continue
                vector.wait_ge(sem_mm, g + 1)
                vector.tensor_scalar(
                    o_sb[t][:, ft, coff : coff + csz],
                    ps[g % NPS][:, :csz],
                    b_sb[:, ft : ft + 1],
                    0.0,
                    add,
                    amax,
                ).then_inc(sem_drv, 1)

        @block.tensor
        def _(tensor):
            tensor.wait_ge(sem_wm, 1)
            for _ in range(NWARM):
                tensor.matmul(
                    wps[:, :], lhsT=wm[:, :P], rhs=wm[:, :], start=True, stop=True
                )
            cur_t = -1
            w_waited = [False] * FT
            for g, t, ft, coff, csz in groups:
                if t != cur_t:
                    tensor.wait_ge(sem_x[t], 16)
                    cur_t = t
                if not w_waited[ft]:
                    tensor.wait_ge(sem_wf[ft], 16)
                    w_waited[ft] = True
                pg = g - NPS  # previous occupant of this PSUM slot
                if pg >= 0:
                    dsem, didx = owner(pg)
                    tensor.wait_ge(dsem, didx + 1)
                slot = ps[g % NPS]
                for kt in range(KT):
                    mm = tensor.matmul(
                        slot[:, :csz],
                        lhsT=w_sb[:, ft, kt],
                        rhs=x_sb[t][:, kt, coff : coff + csz],
                        start=(kt == 0),
                        stop=(kt == KT - 1),
                    )
                mm.then_inc(sem_mm, 1)

    nc.compile()
    return nc


def _get_program(cap: int):
    if cap not in _PROGRAM_CACHE:
        _PROGRAM_CACHE[cap] = _build_program(cap)
    return _PROGRAM_CACHE[cap]


def _route(x, cond_ids, W, b):
    """Host-side routing: group rows by condition, build per-core inputs."""
    x = np.ascontiguousarray(np.asarray(x, dtype=np.float32))
    cond_ids = np.asarray(cond_ids, dtype=np.int32)
    W = np.asarray(W, dtype=np.float32)
    b = np.asarray(b, dtype=np.float32)

    counts = np.bincount(cond_ids, minlength=C)
    # exact cap (8-aligned): tile/matmul free dims may be arbitrary, so no
    # 128-padding — padded rows would be pure wasted PE/DMA work
    cap = max(P, math.ceil(counts.max() / 8) * 8)
    order = np.argsort(cond_ids, kind="stable")
    starts = np.concatenate([[0], np.cumsum(counts)])

    relu_b = np.maximum(b, 0.0)  # [C, D]
    S = relu_b.sum(axis=0)  # [D]

    in_maps = []
    rows_per_core = []
    corrs = []
    for c in range(C):
        rows_c = order[starts[c] : starts[c + 1]]
        rows_per_core.append(rows_c)
        xT_c = np.zeros((D, cap), dtype=np.float16)
        if len(rows_c):
            xT_c[:, : len(rows_c)] = x[rows_c].T
        # w5[p, ft, kt, f2] = W_c[kt*128 + p, ft*128 + f2]
        w5 = np.ascontiguousarray(
            W[c].astype(np.float16).reshape(KT, P, FT, P).transpose(1, 2, 0, 3)
        )
        corrs.append(S - relu_b[c])
        in_maps.append(
            {
                "xt": xT_c,
                "w5": w5,
                "b2": np.ascontiguousarray(b[c].reshape(FT, P).T),
            }
        )
    return in_maps, rows_per_core, corrs, cap


def run(x, cond_ids, W, b, trace: bool = False):
    """Run the kernel; returns (out, BassKernelResults)."""
    try:
        from concourse.bass_utils import run_bass_kernel_spmd
    except ImportError:
        import sys

        sys.path.append("/opt/trn_rl_repo")
        from concourse.bass_utils import run_bass_kernel_spmd

    in_maps, rows_per_core, corrs, cap = _route(x, cond_ids, W, b)
    nc = _get_program(cap)
    res = run_bass_kernel_spmd(
        nc, in_maps, core_ids=list(range(NCORES)), trace=trace
    )

    out = np.empty((len(np.asarray(cond_ids)), D), dtype=np.float32)
    for c in range(C):
        rows_c = rows_per_core[c]
        if len(rows_c):
            out[rows_c] = (
                res.results[c]["yt"][:, : len(rows_c)].T.astype(np.float32)
                + corrs[c][None, :]
            )
    return out, res


def kernel(x, cond_ids, W, b):
    out, _ = run(x, cond_ids, W, b, trace=False)
    return out


# revision 17
# speedup vs baseline: 1.4054x; 1.0073x over previous
"""Trainium2 Bass kernel for nn_ConditionalLayer (MoE-style conditional FC).

Reference semantics (N=16384 rows, D=512 features, C=8 conditions):
    out[n] = sum_c relu( (x[n] * [cond_ids[n]==c]) @ W_c + b_c )
           = relu(x[n] @ W_{c*} + b_{c*}) + corr_{c*}
where c* = cond_ids[n] and corr_c = sum_{c' != c} relu(b_{c'}) is a
per-condition constant vector (masked-out rows still contribute relu(b_c)).

Strategy (expert-parallel, 8 cores == 8 conditions):
  - Host: group rows by condition (argsort), pad to a common CAP, ship core c
    the transposed row-block xT_c = [D, CAP] in fp16 plus W_c (fp16) and b_c.
  - Device: yT = relu(W_c.T-contract xT + b_c), fp16 matmuls on the 128x128
    PE with fp32 PSUM accumulation; PSUM drains split between VectorE
    (tensor_scalar add+max) and ScalarE (activation relu+bias); fp16 out.
  - Host: scatter rows back, adding corr_c in fp32 during the unshard.

The device program is hand-scheduled (raw per-engine Blocks + counting
semaphores, no TileContext): the Tile event-semaphore machinery adds nothing
but overhead for a static pipeline this regular.  fp16 matmuls use 1024-row
moving operands (2 PSUM banks per group) to halve LDWEIGHTS/instruction
overhead.  Warmup matmuls on a junk tile run during the initial DMA window
so the PE HAM clock gate (1.2 GHz cold / 2.4 GHz warm) is released before
real work.  The NEFF epilogue (runtime semaphore-sync chain, ~8us) starts
when the last store lands, so the tail tile is small (128 rows) and the
last stores go on otherwise-idle HWDGE rings.
"""

import math

import numpy as np

N, D, C = 16384, 512, 8
NCORES = 8
P = 128
KT = D // P  # 4 k-tiles
FT = D // P  # 4 output feature tiles
GMAX = 1024  # max x-tile DMA granularity
MMAX = 512  # max moving free dim per matmul (PSUM bank limit)
NPS = 7  # PSUM rotation banks (bank 7 is the warmup target)
NWARM = 24  # warmup matmuls (256 rows each) to release the PE clock gate

_PROGRAM_CACHE: dict = {}


def _x_tile_sizes(cap: int) -> list:
    """DMA granularity: 128 head (starts PE early), <=1024 mids, 128 tail
    so the final store drain is short."""
    if cap >= 1536:
        mid = cap - 256  # head 128 + tail 128
        sizes = [128]
        while mid >= 1024:
            sizes.append(512)
            sizes.append(512)
            mid -= 1024
        if mid:
            sizes.append(mid)
        sizes.append(128)
    else:
        sizes = []
        rem = cap
        while rem:
            s = min(GMAX, rem)
            sizes.append(s)
            rem -= s
    assert sum(sizes) == cap
    return sizes


def _build_program(cap: int):
    import concourse.mybir as mybir
    from concourse import bacc

    f32 = mybir.dt.float32
    f16 = mybir.dt.float16
    relu = mybir.ActivationFunctionType.Relu
    add = mybir.AluOpType.add
    amax = mybir.AluOpType.max

    nc = bacc.Bacc("TRN2", target_bir_lowering=False, debug=False)

    xt = nc.dram_tensor("xt", [D, cap], f16, kind="ExternalInput")
    w5 = nc.dram_tensor("w5", [P, FT, KT, P], f16, kind="ExternalInput")
    b2 = nc.dram_tensor("b2", [P, FT], f32, kind="ExternalInput")
    yt = nc.dram_tensor("yt", [D, cap], f16, kind="ExternalOutput")

    xt_r = xt[:].rearrange("(kt p) r -> p kt r", p=P)  # [128, KT, cap]
    yt_r = yt[:].rearrange("(ft p) r -> p ft r", p=P)  # [128, FT, cap]

    sizes = _x_tile_sizes(cap)
    tiles = []  # (roff, rsz)
    off = 0
    for s in sizes:
        tiles.append((off, s))
        off += s
    ntiles = len(tiles)

    # Static group schedule: one group = one PSUM accumulation (<=512 rows,
    # one ft).  Drains alternate VectorE / ScalarE by parity; PSUM slots
    # rotate over NPS banks.
    groups = []  # (g, tile_idx, ft, coff, csz)
    g = 0
    for t, (roff, rsz) in enumerate(tiles):
        chunks = []
        c0 = 0
        while c0 < rsz:
            cs = min(MMAX, rsz - c0)
            chunks.append((c0, cs))
            c0 += cs
        # ft order 0,2,1,3: ft0/ft1 weight chunks ride the x ring, ft2/ft3
        # the other — interleaving the rings hides each chunk's arrival
        for ft in (0, 2, 1, 3):
            for coff, csz in chunks:
                groups.append((g, t, ft, coff, csz))
                g += 1
    last_group_of_tile = {t: max(g for g, tt, *_ in groups if tt == t) for t in range(ntiles)}

    # All stores ride the SP HWDGE ring: it is idle once the x loads are
    # triggered, and keeping store triggers off the Activation ring keeps
    # ScalarE's drain queue from stalling behind a ~0.7us DMA trigger.

    # SBUF / PSUM allocations
    w_sb = nc.alloc_sbuf_tensor("w_sb", [P, FT, KT, P], f16)
    b_sb = nc.alloc_sbuf_tensor("b_sb", [P, FT], f32)
    wm = nc.alloc_sbuf_tensor("wm", [P, 256], f16)
    x_sb = [
        nc.alloc_sbuf_tensor(f"x_sb{t}", [P, KT, rsz], f16)
        for t, (roff, rsz) in enumerate(tiles)
    ]
    o_sb = [
        nc.alloc_sbuf_tensor(f"o_sb{t}", [P, FT, rsz], f16)
        for t, (roff, rsz) in enumerate(tiles)
    ]
    ps = [nc.alloc_psum_tensor(f"ps{i}", [P, MMAX], f32) for i in range(NPS)]
    wps = nc.alloc_psum_tensor("wps", [P, 256], f32)

    # One semaphore per load DMA: a threshold of 16 means "all 16 SDMA
    # engines finished this transfer".  (Intermediate thresholds on a shared
    # sem are racy: a later transfer's per-engine increments can land while
    # an earlier one is still in flight.)
    sem_x = [nc.alloc_semaphore(f"sem_x{t}") for t in range(ntiles)]
    sem_wf = [nc.alloc_semaphore(f"sem_wf{f}") for f in range(FT)]
    sem_b = nc.alloc_semaphore("sem_b")
    sem_wm = nc.alloc_semaphore("sem_wm")  # warmup tile memset done
    sem_mm = nc.alloc_semaphore("sem_mm")  # PSUM groups finished (stop-MMs)
    sem_drv = nc.alloc_semaphore("sem_drv")  # VectorE drains done
    sem_drs = nc.alloc_semaphore("sem_drs")  # ScalarE drains done
    sem_sts = nc.alloc_semaphore("sem_sts")  # store completions (unwaited)

    def owner(gidx):  # (sem, per-engine index) of group gidx's drain
        return (sem_drv, gidx // 2) if gidx % 2 == 0 else (sem_drs, gidx // 2)

    def drains_done_thresholds(gidx):
        """(v, s) = per-engine drain counts once groups 0..gidx all drained."""
        cnt = gidx + 1
        return (cnt + 1) // 2, cnt // 2

    def emit_store(eng, t, sem_done):
        roff, rsz = tiles[t]
        vthr, sthr = drains_done_thresholds(last_group_of_tile[t])
        eng.wait_ge(sem_drv, vthr)
        eng.wait_ge(sem_drs, sthr)
        eng.dma_start(yt_r[:, :, roff : roff + rsz], o_sb[t][:, :, :]).then_inc(
            sem_done, 16
        )

    with nc.Block(no_gpsimd_drain=True) as block:

        @block.sync
        def _(sync):
            # First wave balanced across the two HWDGE rings: this ring
            # carries x0 + the first two W ft-chunks, the Activation ring
            # carries b + the other two — the first matmul group is gated on
            # whichever ring finishes its share, so neither should hog the
            # whole first wave.
            for t, (roff, rsz) in enumerate(tiles):
                sync.dma_start(
                    x_sb[t][:, :, :], xt_r[:, :, roff : roff + rsz]
                ).then_inc(sem_x[t], 16)
                if t == 0:
                    sync.dma_start(w_sb[:, 0], w5[:, 0]).then_inc(sem_wf[0], 16)
                    sync.dma_start(w_sb[:, 1], w5[:, 1]).then_inc(sem_wf[1], 16)
            for t in range(ntiles):
                emit_store(sync, t, sem_sts)
            # no final wait: the Block-exit InstDrain on this engine blocks
            # until its DMA queues (including these stores) fully complete

        @block.scalar
        def _(scalar):
            # b + second half of W on the Activation HWDGE ring
            scalar.dma_start(b_sb[:, :], b2[:]).then_inc(sem_b, 16)
            scalar.dma_start(w_sb[:, 2], w5[:, 2]).then_inc(sem_wf[2], 16)
            scalar.dma_start(w_sb[:, 3], w5[:, 3]).then_inc(sem_wf[3], 16)
            # odd-parity PSUM drains: relu(z + b), interleaved with this
            # ring's stores so each store issues as soon as its tile drains
            scalar.wait_ge(sem_b, 16)
            for g, t, ft, coff, csz in groups:
                if g % 2 != 1:
                    continue
                scalar.wait_ge(sem_mm, g + 1)
                scalar.activation(
                    o_sb[t][:, ft, coff : coff + csz],
                    ps[g % NPS][:, :csz],
                    relu,
                    bias=b_sb[:, ft : ft + 1],
                ).then_inc(sem_drs, 1)

        @block.vector
        def _(vector):
            vector.memset(wm[:, :], 0.25).then_inc(sem_wm, 1)
            # even-parity PSUM drains: (z + b) max 0
            vector.wait_ge(sem_b, 16)
            for g, t, ft, coff, csz in groups:
                if g % 2 != 0:
                    continue
                vector.wait_ge(sem_mm, g + 1)
                vector.tensor_scalar(
                    o_sb[t][:, ft, coff : coff + csz],
                    ps[g % NPS][:, :csz],
                    b_sb[:, ft : ft + 1],
                    0.0,
                    add,
                    amax,
                ).then_inc(sem_drv, 1)

        @block.tensor
        def _(tensor):
            tensor.wait_ge(sem_wm, 1)
            for _ in range(NWARM):
                tensor.matmul(
                    wps[:, :], lhsT=wm[:, :P], rhs=wm[:, :], start=True, stop=True
                )
            cur_t = -1
            w_waited = [False] * FT
            for g, t, ft, coff, csz in groups:
                if t != cur_t:
                    tensor.wait_ge(sem_x[t], 16)
                    cur_t = t
                if not w_waited[ft]:
                    tensor.wait_ge(sem_wf[ft], 16)
                    w_waited[ft] = True
                pg = g - NPS  # previous occupant of this PSUM slot
                if pg >= 0:
                    dsem, didx = owner(pg)
                    tensor.wait_ge(dsem, didx + 1)
                slot = ps[g % NPS]
                for kt in range(KT):
                    mm = tensor.matmul(
                        slot[:, :csz],
                        lhsT=w_sb[:, ft, kt],
                        rhs=x_sb[t][:, kt, coff : coff + csz],
                        start=(kt == 0),
                        stop=(kt == KT - 1),
                    )
                mm.then_inc(sem_mm, 1)

    nc.compile()
    return nc


def _get_program(cap: int):
    if cap not in _PROGRAM_CACHE:
        _PROGRAM_CACHE[cap] = _build_program(cap)
    return _PROGRAM_CACHE[cap]


def _route(x, cond_ids, W, b):
    """Host-side routing: group rows by condition, build per-core inputs."""
    x = np.ascontiguousarray(np.asarray(x, dtype=np.float32))
    cond_ids = np.asarray(cond_ids, dtype=np.int32)
    W = np.asarray(W, dtype=np.float32)
    b = np.asarray(b, dtype=np.float32)

    counts = np.bincount(cond_ids, minlength=C)
    # exact cap (8-aligned): tile/matmul free dims may be arbitrary, so no
    # 128-padding — padded rows would be pure wasted PE/DMA work
    cap = max(P, math.ceil(counts.max() / 8) * 8)
    order = np.argsort(cond_ids, kind="stable")
    starts = np.concatenate([[0], np.cumsum(counts)])

    relu_b = np.maximum(b, 0.0)  # [C, D]
    S = relu_b.sum(axis=0)  # [D]

    in_maps = []
    rows_per_core = []
    corrs = []
    for c in range(C):
        rows_c = order[starts[c] : starts[c + 1]]
        rows_per_core.append(rows_c)
        xT_c = np.zeros((D, cap), dtype=np.float16)
        if len(rows_c):
            xT_c[:, : len(rows_c)] = x[rows_c].T
        # w5[p, ft, kt, f2] = W_c[kt*128 + p, ft*128 + f2]
        w5 = np.ascontiguousarray(
            W[c].astype(np.float16).reshape(KT, P, FT, P).transpose(1, 2, 0, 3)
        )
        corrs.append(S - relu_b[c])
        in_maps.append(
            {
                "xt": xT_c,
                "w5": w5,
                "b2": np.ascontiguousarray(b[c].reshape(FT, P).T),
            }
        )
    return in_maps, rows_per_core, corrs, cap


def run(x, cond_ids, W, b, trace: bool = False):
    """Run the kernel; returns (out, BassKernelResults)."""
    try:
        from concourse.bass_utils import run_bass_kernel_spmd
    except ImportError:
        import sys

        sys.path.append("/opt/trn_rl_repo")
        from concourse.bass_utils import run_bass_kernel_spmd

    in_maps, rows_per_core, corrs, cap = _route(x, cond_ids, W, b)
    nc = _get_program(cap)
    res = run_bass_kernel_spmd(
        nc, in_maps, core_ids=list(range(NCORES)), trace=trace
    )

    out = np.empty((len(np.asarray(cond_ids)), D), dtype=np.float32)
    for c in range(C):
        rows_c = rows_per_core[c]
        if len(rows_c):
            out[rows_c] = (
                res.results[c]["yt"][:, : len(rows_c)].T.astype(np.float32)
                + corrs[c][None, :]
            )
    return out, res


def kernel(x, cond_ids, W, b):
    out, _ = run(x, cond_ids, W, b, trace=False)
    return out
